# revision 23
# baseline (speedup 1.0000x reference)
"""MobiusLinear Trainium2 kernel (8-core data-parallel SPMD).

Per row x of shape [128]:
    Mx  = x @ W.T
    xn  = ||x||, mn = ||Mx||
    s   = tanh(mn/xn * artanh(xn)) / mn
    y   = s * Mx
    z0  = ((1+2<y,b>+||b||^2) y + (1-||y||^2) b) / (1+2<y,b>+||y||^2 ||b||^2)
    out = projx(z0)

Dataflow per 128-row tile:
  PE transposes x (fp32, 2 cyc/row), ACT evacuates the transpose to bf16,
  PE matmuls xt against bf16 [W^T | W^T b] (1 cyc/row) into PSUM, ACT
  evacuates [Mx | d] to bf16. Norms are per-tile DVE fused square+reduce.
  The per-row transcendental chain runs batched on [128, 64] arrays (ACT
  for Ln/Exp, DVE for tensor_scalar, GpSimd for small tensor_tensor);
  all divisions go through exp(-ln(.)) on ACT. The final combine
  z = P*Mx + Q*b runs on the PE: a block-diagonal-bias matmul puts
  Q*b^T in a PSUM bank (start=True), then per tile a diag(P) stationary
  (one DVE masked tensor_scalar) matmuls against Mx with start=False,
  accumulating P*Mx on top. One ACT copy evacuates z (bf16) and DMA
  stores it; the host upcasts to fp32.
"""

import os
import sys
import functools

import numpy as np

sys.path.insert(0, "/opt/trn_rl_repo")

from contextlib import ExitStack

import concourse.bass as bass
import concourse.tile as tile
from concourse import bacc, mybir
from concourse.bass_utils import run_bass_kernel_spmd

F32 = mybir.dt.float32
BF16 = mybir.dt.bfloat16
AF = mybir.ActivationFunctionType
OP = mybir.AluOpType
AX = mybir.AxisListType

NCORES = 8
B_FULL = 262144
DIN = 128
DOUT = 128
TG = 4            # tiles per PE/PSUM group (512 rows)
CHUNK = 8         # tiles per DMA transfer (1024 rows)
MAXNORM = np.float32(1.0 - 1e-5)
ART_CLIP = np.float32(1.0 - 1e-7)


def _build_body(ctx, tc, nrows, sb, beta, x_d, wtaug_d, ident_d, bigc_d, mask_d, z_d):
    nc = tc.nc
    ntiles = nrows // 128
    assert ntiles % sb == 0
    gpb = sb // TG                      # groups per scalar batch
    nbatch = ntiles // sb

    # ---- constant pools ----
    cpool = ctx.enter_context(tc.tile_pool(name="consts", bufs=1))
    wtaug = cpool.tile([128, 129], BF16, name="wtaug")
    ident = cpool.tile([128, 128], F32, name="ident")
    bigc = cpool.tile([sb // 2, sb * 128], BF16, name="bigc")
    mask = cpool.tile([128, 128], BF16, name="mask")
    nc.scalar.dma_start(out=wtaug[:], in_=wtaug_d)
    nc.scalar.dma_start(out=ident[:], in_=ident_d)
    nc.scalar.dma_start(out=bigc[:], in_=bigc_d)
    nc.scalar.dma_start(out=mask[:], in_=mask_d)

    # ---- working pools ----
    xg_pool = ctx.enter_context(tc.tile_pool(name="xg", bufs=4))     # [128,8,128] f32
    xt_pool = ctx.enter_context(tc.tile_pool(name="xt", bufs=6))     # [128,4,128] bf16
    zt_pool = ctx.enter_context(tc.tile_pool(name="zt", bufs=4))     # [128,8,128] bf16
    junk_pool = ctx.enter_context(tc.tile_pool(name="junk", bufs=6))
    mx_pool = ctx.enter_context(tc.tile_pool(name="mx", bufs=2))
    sc_pool = ctx.enter_context(tc.tile_pool(name="scal", bufs=2))
    fqt_pool = ctx.enter_context(tc.tile_pool(name="fqt", bufs=4))
    dg_pool = ctx.enter_context(tc.tile_pool(name="diag", bufs=12))   # [128,128] bf16

    # PSUM budget (8 banks of 2KB):
    #   ptr (transpose dest) [128,4,128] f32 = 1 bank x2
    #   pm  (matmul dest)    [128,1024]  f32 = 2 banks x2 (two 129-col tiles
    #       packed per bank at 516B offsets -- no bank straddle)
    #   qb  (z accumulate)   [128,512]   f32 = 1 bank x2
    ptr_pool = ctx.enter_context(tc.tile_pool(name="ptr", bufs=2, space="PSUM"))
    pmx_pool = ctx.enter_context(tc.tile_pool(name="pmx", bufs=2, space="PSUM"))
    pqb_pool = ctx.enter_context(tc.tile_pool(name="pqb", bufs=2, space="PSUM"))

    x_r = x_d.rearrange("(c t p) i -> c p t i", p=128, t=CHUNK)
    z_r = z_d.rearrange("(c t p) i -> c p t i", p=128, t=CHUNK)
    z_r4 = z_d.rearrange("(c t p) i -> c p t i", p=128, t=TG)

    for b in range(nbatch):
        # ---------- batch-level scalar arrays [128, sb] ----------
        sx2 = sc_pool.tile([128, sb], F32, name="sx2")
        m2 = sc_pool.tile([128, sb], F32, name="m2")
        mx = mx_pool.tile([128, sb * 129], BF16, name="mx")
        mx3 = mx[:].rearrange("p (j c) -> p j c", c=129)

        # ---------- phase A ----------
        xgs = {}
        for gg in range(gpb):
            g = b * gpb + gg
            j0 = gg * TG

            if gg % 2 == 0:
                xg8 = xg_pool.tile([128, CHUNK, 128], F32, name="xg8")
                nc.sync.dma_start(out=xg8[:], in_=x_r[g // 2])
                xgs[gg] = xg8
            xg8 = xgs[gg - (gg % 2)]
            t0 = (gg % 2) * TG

            ptr = ptr_pool.tile([128, TG, 128], F32, name="ptr", tag="tr")
            for t in range(TG):
                nc.tensor.transpose(ptr[:, t, :], xg8[:, t0 + t, :], ident[:])
            xt = xt_pool.tile([128, TG, 128], BF16, name="xt")
            nc.scalar.activation(xt[:], ptr[:], AF.Copy)

            pm = pmx_pool.tile([128, 1024], F32, name="pm", tag="pm")
            # [128, bank, tile-in-bank, col] view: strides (512, 129, 1)
            pm4 = pm[:].rearrange("p (h x) -> p h x", h=2)[:, :, 0:258].rearrange(
                "p h (t c) -> p h t c", t=2
            )
            for t in range(TG):
                nc.tensor.matmul(
                    pm4[:, t // 2, t % 2, :], xt[:, t, :], wtaug[:],
                    start=True, stop=True,
                )
            # evacuate [Mx | d] to SBUF (bf16) in one wide op
            mxg = mx[:, j0 * 129 : (j0 + TG) * 129].rearrange(
                "p (h t c) -> p h t c", h=2, t=2
            )
            nc.scalar.activation(mxg, pm4, AF.Copy)

            for t in range(TG):
                j = j0 + t
                xjunk = junk_pool.tile([128, 128], F32, name="xjunk")
                nc.vector.affine_mul_reduce(
                    out=xjunk[:],
                    accum_out=sx2[:, j : j + 1],
                    in0=xg8[:, t0 + t, :],
                    in1=xg8[:, t0 + t, :],
                    scale=1.0,
                    bias=0.0,
                )
                mjunk = junk_pool.tile([128, 128], BF16, name="mjunk")
                nc.vector.affine_mul_reduce(
                    out=mjunk[:],
                    accum_out=m2[:, j : j + 1],
                    in0=mx3[:, j, 0:128],
                    in1=mx3[:, j, 0:128],
                    scale=1.0,
                    bias=0.0,
                )

        # ---------- phase B: per-row scalar math, two half-width chains ----------
        # Each [128, sb//2] chain depends only on its half of sx2/m2, so the
        # first chain overlaps phase A's second half and phase C starts earlier.
        hw = sb // 2

        def sc(name):
            return sc_pool.tile([128, sb], F32, name=name)

        Lx, Lm, xn, u, la, lb = sc("Lx"), sc("Lm"), sc("xn"), sc("u"), sc("la"), sc("lb")
        at, dL, ratio, r2, e2 = sc("at"), sc("dL"), sc("ratio"), sc("r2"), sc("e2")
        tnum, Ltn, Ltd, tdf, se, s = (
            sc("tnum"), sc("Ltn"), sc("Ltd"), sc("tdf"), sc("se"), sc("s"))
        xy, twoxy1, cy, s2t, a2, cb = (
            sc("xy"), sc("twoxy1"), sc("cy"), sc("s2t"), sc("a2"), sc("cb"))
        den0, Lden, rden, cys, p, q = (
            sc("den0"), sc("Lden"), sc("rden"), sc("cys"), sc("p"), sc("q"))
        pm2, qd, inner, t7, qq, z2 = (
            sc("pm2"), sc("qd"), sc("inner"), sc("t7"), sc("qq"), sc("z2"))
        Lz, fln, flnc, f, P, fq = (
            sc("Lz"), sc("fln"), sc("flnc"), sc("f"), sc("P"), sc("fq"))

        fqts = {}
        for h in range(2):
            sl = slice(h * hw, (h + 1) * hw)
            d_h = mx3[:, sl, 128]
            nc.scalar.activation(Lx[:, sl], sx2[:, sl], AF.Ln)
            nc.scalar.activation(Lm[:, sl], m2[:, sl], AF.Ln)
            nc.scalar.activation(xn[:, sl], Lx[:, sl], AF.Exp, scale=0.5)
            nc.vector.tensor_scalar_min(u[:, sl], xn[:, sl], float(ART_CLIP))
            nc.scalar.activation(la[:, sl], u[:, sl], AF.Ln, bias=1.0, scale=1.0)
            nc.scalar.activation(lb[:, sl], u[:, sl], AF.Ln, bias=1.0, scale=-1.0)
            nc.gpsimd.tensor_tensor(at[:, sl], la[:, sl], lb[:, sl], OP.subtract)
            nc.gpsimd.tensor_tensor(dL[:, sl], Lm[:, sl], Lx[:, sl], OP.subtract)
            nc.scalar.activation(ratio[:, sl], dL[:, sl], AF.Exp, scale=0.5)
            nc.gpsimd.tensor_tensor(r2[:, sl], ratio[:, sl], at[:, sl], OP.mult)
            nc.scalar.activation(e2[:, sl], r2[:, sl], AF.Exp)
            nc.vector.tensor_scalar_add(tnum[:, sl], e2[:, sl], -1.0)
            nc.scalar.activation(Ltn[:, sl], tnum[:, sl], AF.Ln)
            nc.scalar.activation(Ltd[:, sl], e2[:, sl], AF.Ln, bias=1.0, scale=1.0)
            nc.gpsimd.tensor_tensor(tdf[:, sl], Ltn[:, sl], Ltd[:, sl], OP.subtract)
            nc.vector.scalar_tensor_tensor(
                out=se[:, sl], in0=Lm[:, sl], scalar=-0.5, in1=tdf[:, sl],
                op0=OP.mult, op1=OP.add,
            )
            nc.scalar.activation(s[:, sl], se[:, sl], AF.Exp)
            nc.gpsimd.tensor_tensor(xy[:, sl], s[:, sl], d_h, OP.mult)
            nc.vector.tensor_scalar(
                out=twoxy1[:, sl], in0=xy[:, sl], scalar1=2.0, scalar2=1.0,
                op0=OP.mult, op1=OP.add,
            )
            nc.vector.tensor_scalar_add(cy[:, sl], twoxy1[:, sl], float(beta))
            nc.gpsimd.tensor_tensor(s2t[:, sl], s[:, sl], s[:, sl], OP.mult)
            nc.gpsimd.tensor_tensor(a2[:, sl], s2t[:, sl], m2[:, sl], OP.mult)
            nc.vector.tensor_scalar(
                out=cb[:, sl], in0=a2[:, sl], scalar1=-1.0, scalar2=1.0,
                op0=OP.mult, op1=OP.add,
            )
            nc.vector.scalar_tensor_tensor(
                out=den0[:, sl], in0=a2[:, sl], scalar=float(beta),
                in1=twoxy1[:, sl], op0=OP.mult, op1=OP.add,
            )
            nc.scalar.activation(Lden[:, sl], den0[:, sl], AF.Ln)
            nc.scalar.activation(rden[:, sl], Lden[:, sl], AF.Exp, scale=-1.0)
            nc.gpsimd.tensor_tensor(cys[:, sl], cy[:, sl], s[:, sl], OP.mult)
            nc.gpsimd.tensor_tensor(p[:, sl], cys[:, sl], rden[:, sl], OP.mult)
            nc.gpsimd.tensor_tensor(q[:, sl], cb[:, sl], rden[:, sl], OP.mult)
            nc.gpsimd.tensor_tensor(pm2[:, sl], p[:, sl], m2[:, sl], OP.mult)
            nc.gpsimd.tensor_tensor(qd[:, sl], q[:, sl], d_h, OP.mult)
            nc.vector.scalar_tensor_tensor(
                out=inner[:, sl], in0=qd[:, sl], scalar=2.0, in1=pm2[:, sl],
                op0=OP.mult, op1=OP.add,
            )
            nc.gpsimd.tensor_tensor(t7[:, sl], p[:, sl], inner[:, sl], OP.mult)
            nc.gpsimd.tensor_tensor(qq[:, sl], q[:, sl], q[:, sl], OP.mult)
            nc.vector.scalar_tensor_tensor(
                out=z2[:, sl], in0=qq[:, sl], scalar=float(beta), in1=t7[:, sl],
                op0=OP.mult, op1=OP.add,
            )
            nc.scalar.activation(Lz[:, sl], z2[:, sl], AF.Ln)
            nc.vector.tensor_scalar(
                out=fln[:, sl], in0=Lz[:, sl], scalar1=-0.5,
                scalar2=float(np.log(MAXNORM)), op0=OP.mult, op1=OP.add,
            )
            nc.vector.tensor_scalar_min(flnc[:, sl], fln[:, sl], 0.0)
            nc.scalar.activation(f[:, sl], flnc[:, sl], AF.Exp)
            nc.gpsimd.tensor_tensor(P[:, sl], f[:, sl], p[:, sl], OP.mult)
            nc.gpsimd.tensor_tensor(fq[:, sl], f[:, sl], q[:, sl], OP.mult)

            # transpose this half of fq -> [hw, 128] bf16 for the rank-1 matmul
            pfq = ptr_pool.tile([hw, 128], F32, name="pfq", tag="tr")
            nc.tensor.transpose(pfq[:], fq[:, sl], ident[:])
            fqt_h = fqt_pool.tile([hw, 128], BF16, name=f"fqt{h}")
            nc.scalar.activation(fqt_h[:], pfq[:], AF.Copy)
            fqts[h] = fqt_h

        # ---------- phase C ----------
        zts = {}
        for gg in range(gpb):
            g = b * gpb + gg
            j0 = gg * TG

            qb = pqb_pool.tile([128, TG * 128], F32, name="qb")
            # Q*b^T for 4 tiles via the folded block-diagonal bias constant
            nc.tensor.matmul(
                qb[:], fqts[j0 // (sb // 2)][:],
                bigc[:, j0 * 128 : (j0 + TG) * 128],
                start=True, stop=False, skip_group_check=True,
            )
            # accumulate P*Mx per tile: diag(P_j) as stationary
            for t in range(TG):
                j = j0 + t
                dg = dg_pool.tile([128, 128], BF16, name="dg")
                nc.vector.tensor_scalar(
                    out=dg[:], in0=mask[:], scalar1=P[:, j : j + 1], scalar2=None,
                    op0=OP.mult,
                )
                nc.tensor.matmul(
                    qb[:, t * 128 : (t + 1) * 128], dg[:], mx3[:, j, 0:128],
                    start=False, stop=(t == TG - 1), skip_group_check=True,
                )

            if gg % 2 == 0:
                zt8 = zt_pool.tile([128, CHUNK, 128], BF16, name="zt8")
                zts[gg] = zt8
            zt8 = zts[gg - (gg % 2)]
            t0 = (gg % 2) * TG

            nc.scalar.activation(
                zt8[:, t0 : t0 + TG, :],
                qb[:].rearrange("p (t i) -> p t i", t=TG),
                AF.Copy,
            )
            if b == nbatch - 1:
                # last batch: stream each group out as soon as it is ready
                nc.sync.dma_start(
                    out=z_r4[g], in_=zt8[:, t0 : t0 + TG, :]
                )
            elif gg % 2 == 1:
                nc.sync.dma_start(out=z_r[g // 2], in_=zt8[:])


def _pin_act_tables(arch):
    """Steer every activation this kernel uses into one ACT table set, so the
    whole kernel does a single table load instead of ping-ponging."""
    from concourse import hw_specs

    if os.environ.get("MOBIUS_NO_ACT_PIN"):
        return
    tabs = hw_specs.get_activation_tables(arch)
    target = "natural_log_exp_and_others"
    used = {AF.Ln, AF.Exp, AF.Copy, AF.Square, AF.Identity}
    if target in tabs and used <= tabs[target]:
        for name, s in tabs.items():
            if name != target:
                s -= used


@functools.lru_cache(maxsize=4)
def _build_program(nrows, sb, beta):
    nc = bacc.Bacc(
        "TRN2", target_bir_lowering=False, debug=False, enable_asserts=False
    )
    _pin_act_tables(nc.m.arch)
    x_d = nc.dram_tensor("x", [nrows, DIN], F32, kind="ExternalInput").ap()
    wtaug_d = nc.dram_tensor("wtaug", [128, 129], BF16, kind="ExternalInput").ap()
    ident_d = nc.dram_tensor("ident", [128, 128], F32, kind="ExternalInput").ap()
    bigc_d = nc.dram_tensor("bigc", [sb // 2, sb * 128], BF16, kind="ExternalInput").ap()
    mask_d = nc.dram_tensor("mask", [128, 128], BF16, kind="ExternalInput").ap()
    z_d = nc.dram_tensor("z", [nrows, DOUT], BF16, kind="ExternalOutput").ap()

    with tile.TileContext(nc) as tc:
        with ExitStack() as ctx:
            _build_body(
                ctx, tc, nrows, sb, beta, x_d, wtaug_d, ident_d, bigc_d, mask_d, z_d
            )
    nc.compile()
    return nc


def _make_consts(weight, bias, sb):
    w = np.asarray(weight, dtype=np.float32)
    bvec = np.asarray(bias, dtype=np.float32)
    wtaug = np.zeros((128, 129), dtype=np.float32)
    wtaug[:, :128] = w.T
    wtaug[:, 128] = w.T @ bvec
    ident = np.eye(128, dtype=np.float32)
    bigc = np.zeros((sb // 2, sb * 128), dtype=np.float32)
    for j in range(sb):
        bigc[j % (sb // 2), j * 128 : (j + 1) * 128] = bvec
    mask = np.eye(128, dtype=np.float32)
    beta = float(np.float32(np.dot(bvec.astype(np.float64), bvec.astype(np.float64))))
    return wtaug, ident, bigc, mask, beta


def _bf16(a):
    import jax.numpy as jnp

    return np.asarray(jnp.asarray(a, dtype=jnp.bfloat16))


def _in_maps(x, wtaug, ident, bigc, mask, nrows):
    wtaug16, bigc16, mask16 = _bf16(wtaug), _bf16(bigc), _bf16(mask)
    return [
        {
            "x": x[c * nrows : (c + 1) * nrows],
            "wtaug": wtaug16,
            "ident": ident,
            "bigc": bigc16,
            "mask": mask16,
        }
        for c in range(NCORES)
    ]


def kernel(x, weight, bias, _nrows_per_core=None, _sb=64, _trace=False):
    x = np.ascontiguousarray(np.asarray(x, dtype=np.float32))
    nrows_total = x.shape[0]
    nrows = _nrows_per_core or nrows_total // NCORES
    assert nrows_total == nrows * NCORES

    wtaug, ident, bigc, mask, beta = _make_consts(weight, bias, _sb)
    nc = _build_program(nrows, _sb, beta)

    in_maps = _in_maps(x, wtaug, ident, bigc, mask, nrows)
    res = run_bass_kernel_spmd(nc, in_maps, list(range(NCORES)), trace=_trace)
    out = np.concatenate(
        [np.asarray(res.results[c]["z"]).astype(np.float32) for c in range(NCORES)],
        axis=0,
    )
    kernel._last_results = res
    return out


# revision 27
# speedup vs baseline: 1.0907x; 1.0907x over previous
"""MobiusLinear Trainium2 kernel (8-core data-parallel SPMD).

Per row x of shape [128]:
    Mx  = x @ W.T
    xn  = ||x||, mn = ||Mx||
    s   = tanh(mn/xn * artanh(xn)) / mn
    y   = s * Mx
    z0  = ((1+2<y,b>+||b||^2) y + (1-||y||^2) b) / (1+2<y,b>+||y||^2 ||b||^2)
    out = projx(z0)

Dataflow per 128-row tile:
  PE transposes x (fp32, 2 cyc/row), ACT evacuates the transpose to bf16,
  PE matmuls xt against bf16 [W^T | W^T b] (1 cyc/row) into PSUM, ACT
  evacuates [Mx | d] to bf16. Norms are per-tile DVE fused square+reduce.
  The per-row transcendental chain runs batched on [128, 64] arrays (ACT
  for Ln/Exp, DVE for tensor_scalar, GpSimd for small tensor_tensor);
  all divisions go through exp(-ln(.)) on ACT. The final combine
  z = P*Mx + Q*b runs on the PE: a block-diagonal-bias matmul puts
  Q*b^T in a PSUM bank (start=True), then per tile a diag(P) stationary
  (one DVE masked tensor_scalar) matmuls against Mx with start=False,
  accumulating P*Mx on top. One ACT copy evacuates z (bf16) and DMA
  stores it; the host upcasts to fp32.
"""

import os
import sys
import functools

import numpy as np

sys.path.insert(0, "/opt/trn_rl_repo")

from contextlib import ExitStack

import concourse.bass as bass
import concourse.tile as tile
from concourse import bacc, mybir
from concourse.bass_utils import run_bass_kernel_spmd

F32 = mybir.dt.float32
BF16 = mybir.dt.bfloat16
AF = mybir.ActivationFunctionType
OP = mybir.AluOpType
AX = mybir.AxisListType

NCORES = 8
B_FULL = 262144
DIN = 128
DOUT = 128
TG = 4            # tiles per PE/PSUM group (512 rows)
CHUNK = 8         # tiles per DMA transfer (1024 rows)
MAXNORM = np.float32(1.0 - 1e-5)
ART_CLIP = np.float32(1.0 - 1e-7)


def _build_body(ctx, tc, nrows, sb, beta, x_d, wtaug_d, ident_d, bigc_d, mask_d, z_d):
    nc = tc.nc
    ntiles = nrows // 128
    assert ntiles % sb == 0
    gpb = sb // TG                      # groups per scalar batch
    nbatch = ntiles // sb

    # ---- constant pools ----
    cpool = ctx.enter_context(tc.tile_pool(name="consts", bufs=1))
    wtaug = cpool.tile([128, 129], BF16, name="wtaug")
    ident = cpool.tile([128, 128], F32, name="ident")
    bigc = cpool.tile([sb, sb * 128], BF16, name="bigc")
    mask = cpool.tile([128, 128], BF16, name="mask")
    nc.scalar.dma_start(out=wtaug[:], in_=wtaug_d)
    nc.scalar.dma_start(out=ident[:], in_=ident_d)
    nc.scalar.dma_start(out=bigc[:], in_=bigc_d)
    nc.scalar.dma_start(out=mask[:], in_=mask_d)

    # ---- working pools ----
    xg_pool = ctx.enter_context(tc.tile_pool(name="xg", bufs=4))     # [128,8,128] f32
    xt_pool = ctx.enter_context(tc.tile_pool(name="xt", bufs=6))     # [128,4,128] bf16
    zt_pool = ctx.enter_context(tc.tile_pool(name="zt", bufs=4))     # [128,8,128] bf16
    junk_pool = ctx.enter_context(tc.tile_pool(name="junk", bufs=6))
    mx_pool = ctx.enter_context(tc.tile_pool(name="mx", bufs=2))
    sc_pool = ctx.enter_context(tc.tile_pool(name="scal", bufs=2))
    fqt_pool = ctx.enter_context(tc.tile_pool(name="fqt", bufs=2))
    dg_pool = ctx.enter_context(tc.tile_pool(name="diag", bufs=12))   # [128,128] bf16

    # PSUM budget (8 banks of 2KB):
    #   ptr (transpose dest) [128,4,128] f32 = 1 bank x2
    #   pm  (matmul dest)    [128,1024]  f32 = 2 banks x2 (two 129-col tiles
    #       packed per bank at 516B offsets -- no bank straddle)
    #   qb  (z accumulate)   [128,512]   f32 = 1 bank x2
    ptr_pool = ctx.enter_context(tc.tile_pool(name="ptr", bufs=2, space="PSUM"))
    pmx_pool = ctx.enter_context(tc.tile_pool(name="pmx", bufs=2, space="PSUM"))
    pqb_pool = ctx.enter_context(tc.tile_pool(name="pqb", bufs=2, space="PSUM"))

    x_r = x_d.rearrange("(c t p) i -> c p t i", p=128, t=CHUNK)
    z_r = z_d.rearrange("(c t p) i -> c p t i", p=128, t=CHUNK)
    z_r4 = z_d.rearrange("(c t p) i -> c p t i", p=128, t=TG)

    for b in range(nbatch):
        # ---------- batch-level scalar arrays [128, sb] ----------
        sx2 = sc_pool.tile([128, sb], F32, name="sx2")
        m2 = sc_pool.tile([128, sb], F32, name="m2")
        mx = mx_pool.tile([128, sb * 129], BF16, name="mx")
        mx3 = mx[:].rearrange("p (j c) -> p j c", c=129)

        # ---------- phase A ----------
        xgs = {}
        for gg in range(gpb):
            g = b * gpb + gg
            j0 = gg * TG

            if gg % 2 == 0:
                xg8 = xg_pool.tile([128, CHUNK, 128], F32, name="xg8")
                nc.sync.dma_start(out=xg8[:], in_=x_r[g // 2])
                xgs[gg] = xg8
            xg8 = xgs[gg - (gg % 2)]
            t0 = (gg % 2) * TG

            ptr = ptr_pool.tile([128, TG, 128], F32, name="ptr", tag="tr")
            for t in range(TG):
                nc.tensor.transpose(ptr[:, t, :], xg8[:, t0 + t, :], ident[:])
            xt = xt_pool.tile([128, TG, 128], BF16, name="xt")
            nc.scalar.activation(xt[:], ptr[:], AF.Copy)

            pm = pmx_pool.tile([128, 1024], F32, name="pm", tag="pm")
            # [128, bank, tile-in-bank, col] view: strides (512, 129, 1)
            pm4 = pm[:].rearrange("p (h x) -> p h x", h=2)[:, :, 0:258].rearrange(
                "p h (t c) -> p h t c", t=2
            )
            for t in range(TG):
                nc.tensor.matmul(
                    pm4[:, t // 2, t % 2, :], xt[:, t, :], wtaug[:],
                    start=True, stop=True,
                )
            # evacuate [Mx | d] to SBUF (bf16) in one wide op
            mxg = mx[:, j0 * 129 : (j0 + TG) * 129].rearrange(
                "p (h t c) -> p h t c", h=2, t=2
            )
            nc.scalar.activation(mxg, pm4, AF.Copy)

            for t in range(TG):
                j = j0 + t
                xjunk = junk_pool.tile([128, 128], F32, name="xjunk")
                nc.vector.affine_mul_reduce(
                    out=xjunk[:],
                    accum_out=sx2[:, j : j + 1],
                    in0=xg8[:, t0 + t, :],
                    in1=xg8[:, t0 + t, :],
                    scale=1.0,
                    bias=0.0,
                )
                mjunk = junk_pool.tile([128, 128], BF16, name="mjunk")
                nc.vector.affine_mul_reduce(
                    out=mjunk[:],
                    accum_out=m2[:, j : j + 1],
                    in0=mx3[:, j, 0:128],
                    in1=mx3[:, j, 0:128],
                    scale=1.0,
                    bias=0.0,
                )

        # ---------- phase B: batched per-row scalar math on [128, sb] ----------
        def sc(name):
            return sc_pool.tile([128, sb], F32, name=name)

        d_ap = mx3[:, :, 128]  # [128, sb] strided view of <Mx, bias> (bf16)

        # All transcendentals via Ln/Exp (single ACT table set):
        #   xn    = exp(0.5 ln sx2)
        #   s     = tanh(r2/2)/mn = exp(ln(e2-1) - ln(e2+1) - 0.5 ln m2)
        #   1/den = exp(-ln den)
        Lx = sc("Lx")
        nc.scalar.activation(Lx[:], sx2[:], AF.Ln)
        Lm = sc("Lm")
        nc.scalar.activation(Lm[:], m2[:], AF.Ln)
        xn = sc("xn")
        nc.scalar.activation(xn[:], Lx[:], AF.Exp, scale=0.5)
        u = sc("u")
        nc.vector.tensor_scalar_min(u[:], xn[:], float(ART_CLIP))
        la = sc("la")
        nc.scalar.activation(la[:], u[:], AF.Ln, bias=1.0, scale=1.0)
        lb = sc("lb")
        nc.scalar.activation(lb[:], u[:], AF.Ln, bias=1.0, scale=-1.0)
        # at = la - lb = 2*artanh(u)
        at = sc("at")
        nc.gpsimd.tensor_tensor(at[:], la[:], lb[:], OP.subtract)
        dL = sc("dL")
        nc.gpsimd.tensor_tensor(dL[:], Lm[:], Lx[:], OP.subtract)
        ratio = sc("ratio")
        nc.scalar.activation(ratio[:], dL[:], AF.Exp, scale=0.5)
        # r2 = ratio * at = 2 * (mn/xn) * artanh(xn)
        r2 = sc("r2")
        nc.gpsimd.tensor_tensor(r2[:], ratio[:], at[:], OP.mult)
        e2 = sc("e2")
        nc.scalar.activation(e2[:], r2[:], AF.Exp)
        tnum = sc("tnum")
        nc.vector.tensor_scalar_add(tnum[:], e2[:], -1.0)
        Ltn = sc("Ltn")
        nc.scalar.activation(Ltn[:], tnum[:], AF.Ln)
        Ltd = sc("Ltd")
        nc.scalar.activation(Ltd[:], e2[:], AF.Ln, bias=1.0, scale=1.0)
        tdf = sc("tdf")
        nc.gpsimd.tensor_tensor(tdf[:], Ltn[:], Ltd[:], OP.subtract)
        se = sc("se")
        nc.vector.scalar_tensor_tensor(
            out=se[:], in0=Lm[:], scalar=-0.5, in1=tdf[:],
            op0=OP.mult, op1=OP.add,
        )
        s = sc("s")
        nc.scalar.activation(s[:], se[:], AF.Exp)

        xy = sc("xy")
        nc.gpsimd.tensor_tensor(xy[:], s[:], d_ap, OP.mult)
        twoxy1 = sc("twoxy1")
        nc.vector.tensor_scalar(
            out=twoxy1[:], in0=xy[:], scalar1=2.0, scalar2=1.0,
            op0=OP.mult, op1=OP.add,
        )
        cy = sc("cy")
        nc.vector.tensor_scalar_add(cy[:], twoxy1[:], float(beta))
        s2t = sc("s2t")
        nc.gpsimd.tensor_tensor(s2t[:], s[:], s[:], OP.mult)
        a2 = sc("a2")
        nc.gpsimd.tensor_tensor(a2[:], s2t[:], m2[:], OP.mult)
        cb = sc("cb")
        nc.vector.tensor_scalar(
            out=cb[:], in0=a2[:], scalar1=-1.0, scalar2=1.0,
            op0=OP.mult, op1=OP.add,
        )
        den0 = sc("den0")
        nc.vector.scalar_tensor_tensor(
            out=den0[:], in0=a2[:], scalar=float(beta), in1=twoxy1[:],
            op0=OP.mult, op1=OP.add,
        )
        # den0 = 1 + 2<y,b> + |y|^2|b|^2 ~ 1 for ball-interior data
        Lden = sc("Lden")
        nc.scalar.activation(Lden[:], den0[:], AF.Ln)
        rden = sc("rden")
        nc.scalar.activation(rden[:], Lden[:], AF.Exp, scale=-1.0)

        cys = sc("cys")
        nc.gpsimd.tensor_tensor(cys[:], cy[:], s[:], OP.mult)
        p = sc("p")
        nc.gpsimd.tensor_tensor(p[:], cys[:], rden[:], OP.mult)
        q = sc("q")
        nc.gpsimd.tensor_tensor(q[:], cb[:], rden[:], OP.mult)

        # ||z0||^2 = p^2 m2 + 2 p q d + q^2 beta
        pm2 = sc("pm2")
        nc.gpsimd.tensor_tensor(pm2[:], p[:], m2[:], OP.mult)
        qd = sc("qd")
        nc.gpsimd.tensor_tensor(qd[:], q[:], d_ap, OP.mult)
        inner = sc("inner")
        nc.vector.scalar_tensor_tensor(
            out=inner[:], in0=qd[:], scalar=2.0, in1=pm2[:],
            op0=OP.mult, op1=OP.add,
        )
        t7 = sc("t7")
        nc.gpsimd.tensor_tensor(t7[:], p[:], inner[:], OP.mult)
        qq = sc("qq")
        nc.gpsimd.tensor_tensor(qq[:], q[:], q[:], OP.mult)
        z2 = sc("z2")
        nc.vector.scalar_tensor_tensor(
            out=z2[:], in0=qq[:], scalar=float(beta), in1=t7[:],
            op0=OP.mult, op1=OP.add,
        )
        # f = min(1, maxnorm/||z||) = exp(min(0, ln(maxnorm) - 0.5 ln z2))
        Lz = sc("Lz")
        nc.scalar.activation(Lz[:], z2[:], AF.Ln)
        fln = sc("fln")
        nc.vector.tensor_scalar(
            out=fln[:], in0=Lz[:], scalar1=-0.5, scalar2=float(np.log(MAXNORM)),
            op0=OP.mult, op1=OP.add,
        )
        flnc = sc("flnc")
        nc.vector.tensor_scalar_min(flnc[:], fln[:], 0.0)
        f = sc("f")
        nc.scalar.activation(f[:], flnc[:], AF.Exp)

        P = sc("P")
        nc.gpsimd.tensor_tensor(P[:], f[:], p[:], OP.mult)
        fq = sc("fq")
        nc.gpsimd.tensor_tensor(fq[:], f[:], q[:], OP.mult)

        # transpose fq [128, sb] -> [sb, 128] (bf16) for the rank-1 matmul
        pfq = ptr_pool.tile([sb, 128], F32, name="pfq", tag="tr")
        nc.tensor.transpose(pfq[:], fq[:], ident[:])
        fqt = fqt_pool.tile([sb, 128], BF16, name="fqt")
        nc.scalar.activation(fqt[:], pfq[:], AF.Copy)

        # ---------- phase C ----------
        zts = {}
        for gg in range(gpb):
            g = b * gpb + gg
            j0 = gg * TG

            qb = pqb_pool.tile([128, TG * 128], F32, name="qb")
            # Q*b^T for 4 tiles via the block-diagonal bias constant
            nc.tensor.matmul(
                qb[:], fqt[:], bigc[:, j0 * 128 : (j0 + TG) * 128],
                start=True, stop=False, skip_group_check=True,
            )
            # accumulate P*Mx per tile: diag(P_j) as stationary
            for t in range(TG):
                j = j0 + t
                dg = dg_pool.tile([128, 128], BF16, name="dg")
                nc.vector.tensor_scalar(
                    out=dg[:], in0=mask[:], scalar1=P[:, j : j + 1], scalar2=None,
                    op0=OP.mult,
                )
                nc.tensor.matmul(
                    qb[:, t * 128 : (t + 1) * 128], dg[:], mx3[:, j, 0:128],
                    start=False, stop=(t == TG - 1), skip_group_check=True,
                )

            if gg % 2 == 0:
                zt8 = zt_pool.tile([128, CHUNK, 128], BF16, name="zt8")
                zts[gg] = zt8
            zt8 = zts[gg - (gg % 2)]
            t0 = (gg % 2) * TG

            nc.scalar.activation(
                zt8[:, t0 : t0 + TG, :],
                qb[:].rearrange("p (t i) -> p t i", t=TG),
                AF.Copy,
            )
            # stream each group out as soon as it is ready
            nc.sync.dma_start(out=z_r4[g], in_=zt8[:, t0 : t0 + TG, :])


def _pin_act_tables(arch):
    """Steer every activation this kernel uses into one ACT table set, so the
    whole kernel does a single table load instead of ping-ponging."""
    from concourse import hw_specs

    if os.environ.get("MOBIUS_NO_ACT_PIN"):
        return
    tabs = hw_specs.get_activation_tables(arch)
    target = "natural_log_exp_and_others"
    used = {AF.Ln, AF.Exp, AF.Copy, AF.Square, AF.Identity}
    if target in tabs and used <= tabs[target]:
        for name, s in tabs.items():
            if name != target:
                s -= used


@functools.lru_cache(maxsize=4)
def _build_program(nrows, sb, beta):
    nc = bacc.Bacc(
        "TRN2", target_bir_lowering=False, debug=False, enable_asserts=False
    )
    _pin_act_tables(nc.m.arch)
    x_d = nc.dram_tensor("x", [nrows, DIN], F32, kind="ExternalInput").ap()
    wtaug_d = nc.dram_tensor("wtaug", [128, 129], BF16, kind="ExternalInput").ap()
    ident_d = nc.dram_tensor("ident", [128, 128], F32, kind="ExternalInput").ap()
    bigc_d = nc.dram_tensor("bigc", [sb, sb * 128], BF16, kind="ExternalInput").ap()
    mask_d = nc.dram_tensor("mask", [128, 128], BF16, kind="ExternalInput").ap()
    z_d = nc.dram_tensor("z", [nrows, DOUT], BF16, kind="ExternalOutput").ap()

    with tile.TileContext(nc) as tc:
        with ExitStack() as ctx:
            _build_body(
                ctx, tc, nrows, sb, beta, x_d, wtaug_d, ident_d, bigc_d, mask_d, z_d
            )
    nc.compile()
    return nc


def _make_consts(weight, bias, sb):
    w = np.asarray(weight, dtype=np.float32)
    bvec = np.asarray(bias, dtype=np.float32)
    wtaug = np.zeros((128, 129), dtype=np.float32)
    wtaug[:, :128] = w.T
    wtaug[:, 128] = w.T @ bvec
    ident = np.eye(128, dtype=np.float32)
    bigc = np.zeros((sb, sb * 128), dtype=np.float32)
    for j in range(sb):
        bigc[j, j * 128 : (j + 1) * 128] = bvec
    mask = np.eye(128, dtype=np.float32)
    beta = float(np.float32(np.dot(bvec.astype(np.float64), bvec.astype(np.float64))))
    return wtaug, ident, bigc, mask, beta


def _bf16(a):
    import jax.numpy as jnp

    return np.asarray(jnp.asarray(a, dtype=jnp.bfloat16))


def _in_maps(x, wtaug, ident, bigc, mask, nrows):
    wtaug16, bigc16, mask16 = _bf16(wtaug), _bf16(bigc), _bf16(mask)
    return [
        {
            "x": x[c * nrows : (c + 1) * nrows],
            "wtaug": wtaug16,
            "ident": ident,
            "bigc": bigc16,
            "mask": mask16,
        }
        for c in range(NCORES)
    ]


def kernel(x, weight, bias, _nrows_per_core=None, _sb=64, _trace=False):
    x = np.ascontiguousarray(np.asarray(x, dtype=np.float32))
    nrows_total = x.shape[0]
    nrows = _nrows_per_core or nrows_total // NCORES
    assert nrows_total == nrows * NCORES

    wtaug, ident, bigc, mask, beta = _make_consts(weight, bias, _sb)
    nc = _build_program(nrows, _sb, beta)

    in_maps = _in_maps(x, wtaug, ident, bigc, mask, nrows)
    res = run_bass_kernel_spmd(nc, in_maps, list(range(NCORES)), trace=_trace)
    out = np.concatenate(
        [np.asarray(res.results[c]["z"]).astype(np.float32) for c in range(NCORES)],
        axis=0,
    )
    kernel._last_results = res
    return out
